# revision 26
# baseline (speedup 1.0000x reference)
"""Tensor-parallel attention forward (B=4, S=512, D=4096, H=32, HKV=8, HD=128,
START=512) on 8 TRN2 NeuronCores.

Sharding (chosen): TP over heads. Each core c owns q-heads 4c..4c+3 (wq rows
512c:512c+512), kv-head c (wk/wv rows 128c:128c+128, cache slice c), and
output columns 512c:512c+512 (wo rows 512c:512c+512). x is replicated. After
local attention, per-core attention outputs (head-sharded) are AllGathered
(bf16, split in two per token block for earlier comm start) and each core
computes its own 512-column slice of the output projection — no reduction
collective needed. The host concatenates the 8 column slices.

Host-side layout prep (part of sharding): operands are pre-transposed so the
contraction dim (model dim d / feature dim e) lands on SBUF partitions with
natural-stride DMA, pre-cast to bf16 (the on-device compute precision — this
halves HBM traffic), and RoPE pair dims of wq/wk/cache_k are pre-permuted to
[evens, odds] so the on-chip rotation is two contiguous 64-partition blocks.

Compute: bf16 matmuls (fp32 PSUM accumulate), fp32 softmax denominators.
Causal structure: key-tile kt >= 4 only attends to queries s >= 128*(kt-4);
matmul N, exp and denominator work are trimmed accordingly, and only the
128-wide diagonal block needs the affine predicate fill.
"""
import math

import numpy as np
import ml_dtypes

import concourse.mybir as mybir
from concourse import bass
from concourse.tile import TileContext
from concourse.bass_utils import run_bass_kernel_spmd

F32 = mybir.dt.float32
BF16 = mybir.dt.bfloat16
NPBF16 = ml_dtypes.bfloat16

NCORES = 8
B, S, D = 4, 512, 4096
H, HKV, HD = 32, 8, 128
START = 512
T = START + S          # 1024 total kv length
NT = B * S             # 2048 tokens
NH = H // NCORES       # 4 local q heads
EL = NH * HD           # 512 local e width
DT = D // 128          # 32 d-tiles
KT = T // 128          # 8 k-tiles
NKC = START // 128     # 4 cached k-tiles
SCALE = 1.0 / math.sqrt(HD)

# RoPE pair permutation: head-dim reordered to [evens, odds]
PERM = np.concatenate([np.arange(0, HD, 2), np.arange(1, HD, 2)])

SPLIT_AG = True   # kept for compat; NGR is authoritative
NGR = 2           # gathers per token block (1, 2, or 4)
PIPE_DEPTH = 1    # token blocks between a gather and its output projection
PT_BUFS = 3       # probability-tile double/triple buffering
TE_BUFS = 2       # exp-staging tiles for the masked diagonal
DUP_DVE = False   # diagnostic: double rope DVE work
DUP_ACT = False   # diagnostic: double exp work
DUP_POOL = False  # diagnostic: double affine_select work
DUP_COLL = False  # diagnostic: double collectives

_counter = [0]


def _dedup_ldweights(nc):
    """Drop InstLdweights whose stationary AP is identical to the previous
    PE weight load (weights persist in the PE array across matmuls)."""
    removed = 0
    for f in nc.m.functions:
        for blk in f.blocks:
            last_sig = None
            keep = []
            for inst in blk.instructions:
                tn = type(inst).__name__
                if tn == "InstLdweights":
                    sig = (str(inst.ins[0])
                           + str(getattr(inst, "tile_position", None))
                           + str(getattr(inst, "tile_size", None)))
                    if sig == last_sig and not (inst.sync_info and inst.sync_info.on_wait):
                        removed += 1
                        continue
                    last_sig = sig
                elif tn == "InstMatmult":
                    # f32 matmuls stay self-loading (no split LDW) and
                    # clobber the PE weight array; transpose-mode matmuls
                    # change array state too
                    try:
                        if getattr(inst, "is_transpose", False) or \
                                "float32" in str(inst.ins[1].dtype):
                            last_sig = None
                    except Exception:
                        last_sig = None
                elif getattr(inst, "engine", None) == mybir.EngineType.PE:
                    last_sig = None
                keep.append(inst)
            blk.instructions = keep
    return removed


def _split_excess_waits(nc, cap: int = 1):
    """This walrus build rejects instructions with >1 sync waits; split the
    extras into leading no-ops on the same engine."""
    for f in nc.m.functions:
        for blk in f.blocks:
            insts = blk.instructions
            i = 0
            while i < len(insts):
                inst = insts[i]
                si = inst.sync_info
                if si is not None and si.on_wait is not None and len(si.on_wait) > cap:
                    waits = list(si.on_wait)
                    extra, keep = waits[:-cap], waits[-cap:]
                    nops = []
                    for j in range(0, len(extra), cap):
                        _counter[0] += 1
                        nops.append(mybir.InstNoOp(
                            name=f"waitsplit-{_counter[0]}",
                            engine=inst.engine, ins=[], outs=[],
                            sync_info=mybir.SyncInfo(
                                on_wait=extra[j:j + cap], on_update=[]),
                        ))
                    inst.sync_info = mybir.SyncInfo(
                        on_wait=keep, on_update=list(si.on_update or []))
                    for k, nop in enumerate(nops):
                        insts.insert(i + k, nop)
                    i += len(nops)
                i += 1


def build_nc(iters: int = 1):
    nc = bass.Bass(num_devices=NCORES)

    xT = nc.declare_dram_parameter("xT", [D, NT], BF16, isOutput=False)
    wqT = nc.declare_dram_parameter("wqT", [D, EL], BF16, isOutput=False)
    wkT = nc.declare_dram_parameter("wkT", [D, HD], BF16, isOutput=False)
    wvT = nc.declare_dram_parameter("wvT", [D, HD], BF16, isOutput=False)
    woT = nc.declare_dram_parameter("woT", [D, EL], BF16, isOutput=False)
    ckT = nc.declare_dram_parameter("ckT", [B, HD, START], BF16, isOutput=False)
    cv = nc.declare_dram_parameter("cv", [B, START, HD], BF16, isOutput=False)
    cosT = nc.declare_dram_parameter("cosT", [HD // 2, S], BF16, isOutput=False)
    sinT = nc.declare_dram_parameter("sinT", [HD // 2, S], BF16, isOutput=False)
    out = nc.declare_dram_parameter("out", [EL, NT], F32, isOutput=True)

    ngr = NGR
    hpg = NH // ngr  # heads per gather group
    ag_in = [[nc.dram_tensor(f"ag_in_{b}_{g}", [hpg * HD, S], BF16)
              for g in range(ngr)] for b in range(B)]
    ag_out = [[nc.dram_tensor(f"ag_out_{b}_{g}", [NCORES * hpg * HD, S], BF16,
                              addr_space="Shared") for g in range(ngr)]
              for b in range(B)]

    rg = [list(range(NCORES))]

    with TileContext(nc) as tc:
        with (
            tc.tile_pool(name="wpool", bufs=1) as wpool,
            tc.tile_pool(name="cpool", bufs=1) as cpool,
            tc.tile_pool(name="xpool", bufs=2) as xpool,
            tc.tile_pool(name="qkv", bufs=2) as qkv,
            tc.tile_pool(name="work", bufs=2) as work,
            tc.tile_pool(name="denp", bufs=1) as denp,
            tc.tile_pool(name="ptp", bufs=PT_BUFS) as ptp,
            tc.tile_pool(name="tep", bufs=TE_BUFS) as tep,
            tc.tile_pool(name="rope", bufs=2) as ropep,
            tc.tile_pool(name="atp", bufs=4) as atp,
            tc.tile_pool(name="gath", bufs=3) as gath,
            tc.tile_pool(name="ps", bufs=2, space="PSUM") as ps,
            tc.tile_pool(name="pspv", bufs=4, space="PSUM") as pspv,
        ):
            # ---- preamble (split + ordered so the first Q matmuls start early)
            wq_s = wpool.tile([128, DT, EL], BF16, tag="wq")
            for q in range(4):
                nc.scalar.dma_start(
                    out=wq_s[:, 8 * q:8 * q + 8, :],
                    in_=wqT[:, :].rearrange("(i p) e -> p i e", p=128)[:, 8 * q:8 * q + 8, :])
            wk_s = wpool.tile([128, DT, HD], BF16, tag="wk")
            nc.scalar.dma_start(out=wk_s[:, :, :],
                                in_=wkT[:, :].rearrange("(i p) e -> p i e", p=128))
            wv_s = wpool.tile([128, DT, HD], BF16, tag="wv")
            nc.scalar.dma_start(out=wv_s[:, :, :],
                                in_=wvT[:, :].rearrange("(i p) e -> p i e", p=128))
            cos_s = cpool.tile([64, S], BF16, tag="cos")
            nc.scalar.dma_start(out=cos_s[:, :], in_=cosT[:, :])
            sin_s = cpool.tile([64, S], BF16, tag="sin")
            nc.scalar.dma_start(out=sin_s[:, :], in_=sinT[:, :])
            ones_m = cpool.tile([128, 128], BF16, tag="onm")
            nc.vector.memset(ones_m[:, :], 1.0)
            # causal mask for the diagonal key tile: maskd[p, :, j] = (j >= p)
            maskd = cpool.tile([128, 2, 128], BF16, tag="maskd")
            nc.vector.memset(maskd[:, :, :], 1.0)
            nc.gpsimd.affine_select(
                out=maskd[:, :, :], in_=maskd[:, :, :],
                pattern=[[0, 2], [1, 128]],
                compare_op=mybir.AluOpType.is_ge,
                fill=0.0, base=0, channel_multiplier=-1)
            # wo is not needed until the first output projection — load late
            wo_s = wpool.tile([128, DT, EL], BF16, tag="wo")

            def load_wo():
                for q in range(4):
                    nc.scalar.dma_start(
                        out=wo_s[:, 8 * q:8 * q + 8, :],
                        in_=woT[:, :].rearrange("(i p) e -> p i e", p=128)[:, 8 * q:8 * q + 8, :])

            def rope(dst_a, dst_b, src):
                """dst = rotate(src); src [128, S] PSUM f32 with partitions
                [evens(a) 0:64, odds(b) 64:128]; dst bf16 [64, S] slices."""
                for _dup in range(2 if DUP_DVE else 1):
                    _rope1(dst_a, dst_b, src)

            def _rope1(dst_a, dst_b, src):
                a, bb = src[0:64, :], src[64:128, :]
                t1 = ropep.tile([64, S], BF16, tag="rt1")
                t2 = ropep.tile([64, S], BF16, tag="rt2")
                nc.vector.tensor_tensor(out=t1[:, :], in0=a, in1=cos_s[:, :],
                                        op=mybir.AluOpType.mult)
                nc.vector.tensor_tensor(out=t2[:, :], in0=bb, in1=sin_s[:, :],
                                        op=mybir.AluOpType.mult)
                nc.vector.tensor_tensor(out=dst_a, in0=t1[:, :], in1=t2[:, :],
                                        op=mybir.AluOpType.subtract)
                t3 = ropep.tile([64, S], BF16, tag="rt3")
                t4 = ropep.tile([64, S], BF16, tag="rt4")
                nc.vector.tensor_tensor(out=t3[:, :], in0=a, in1=sin_s[:, :],
                                        op=mybir.AluOpType.mult)
                nc.vector.tensor_tensor(out=t4[:, :], in0=bb, in1=cos_s[:, :],
                                        op=mybir.AluOpType.mult)
                nc.vector.tensor_tensor(out=dst_b, in0=t3[:, :], in1=t4[:, :],
                                        op=mybir.AluOpType.add)

            def emit_wo(b):
                """Output projection for block b from the gathers. Gather g's
                tile index i covers e-tile 4*(i//hpg) + hpg*g + i%hpg. The
                gathered activations are read back as half-gather 1MB HWDGE
                DMAs on the sync ring (prefetchable during the next block)."""
                ps_y = [pspv.tile([128, S], F32, tag="pspv", name=f"psy{b}_{dj}")
                        for dj in range(4)]
                nchunk = NCORES * hpg  # tiles per gather
                for g in range(ngr):
                    src = ag_out[b][g][:, :].rearrange("(i p) q -> p i q", p=128)
                    for half in range(2):
                        i0 = (nchunk // 2) * half
                        ag_t = gath.tile([128, nchunk // 2, S], BF16, tag="agt")
                        nc.sync.dma_start(
                            out=ag_t[:, :, :],
                            in_=src[:, i0:i0 + nchunk // 2, :])
                        for i2 in range(nchunk // 2):
                            i = i0 + i2
                            c, t2 = divmod(i, hpg)
                            e = 4 * c + hpg * g + t2
                            for dj in range(4):
                                nc.tensor.matmul(
                                    ps_y[dj][:, :],
                                    wo_s[:, e, 128 * dj:128 * dj + 128],
                                    ag_t[:, i2, :],
                                    start=(g == 0 and i == 0),
                                    stop=(g == ngr - 1 and i == nchunk - 1))
                for dj in range(4):
                    yt = work.tile([128, S], F32, tag="yt")
                    nc.vector.tensor_copy(out=yt[:, :], in_=ps_y[dj][:, :])
                    nc.gpsimd.dma_start(
                        out=out[128 * dj:128 * dj + 128, S * b:S * b + S],
                        in_=yt[:, :])

            pending = []
            for it in range(iters):
                for b in range(B):
                    # ---- loads for this token block (= batch b) ----
                    xt0 = xpool.tile([128, DT // 2, S], BF16, tag="xt")
                    xt1 = xpool.tile([128, DT // 2, S], BF16, tag="xt")
                    xsrc = xT[:, S * b:S * b + S].rearrange("(i p) t -> p i t", p=128)
                    for hh in range(2):
                        nc.gpsimd.dma_start(out=xt0[:, 8 * hh:8 * hh + 8, :],
                                            in_=xsrc[:, 8 * hh:8 * hh + 8, :])
                    for hh in range(2):
                        nc.gpsimd.dma_start(out=xt1[:, 8 * hh:8 * hh + 8, :],
                                            in_=xsrc[:, 16 + 8 * hh:16 + 8 * hh + 8, :])

                    def xt(i):
                        return (xt0 if i < DT // 2 else xt1)[:, i % (DT // 2), :]

                    kT_b = qkv.tile([128, T], BF16, tag="kT")
                    nc.sync.dma_start(out=kT_b[:, 0:START], in_=ckT[b])
                    v_b = qkv.tile([128, KT, HD], BF16, tag="v")
                    nc.sync.dma_start(
                        out=v_b[:, 0:NKC, :],
                        in_=cv[b].rearrange("(kt p) dv -> p kt dv", p=128))
                    qT_b = qkv.tile([128, NH, S], BF16, tag="qT")

                    # ---- Q projection + rope (per local head) ----
                    for j in range(NH):
                        ps_q2 = ps.tile([128, 2, S], F32, tag="ps", name=f"psq{b}_{j}")
                        ps_q = ps_q2[:, 0, :]
                        for i in range(DT):
                            nc.tensor.matmul(
                                ps_q, wq_s[:, i, 128 * j:128 * j + 128],
                                xt(i), start=(i == 0), stop=(i == DT - 1))
                        rope(qT_b[0:64, j, :], qT_b[64:128, j, :], ps_q)

                    # ---- K projection + rope ----
                    ps_k2 = ps.tile([128, 2, S], F32, tag="ps")
                    ps_k = ps_k2[:, 0, :]
                    for i in range(DT):
                        nc.tensor.matmul(ps_k, wk_s[:, i, :], xt(i),
                                         start=(i == 0), stop=(i == DT - 1))
                    rope(kT_b[0:64, START:T], kT_b[64:128, START:T], ps_k)

                    # ---- V projection as V^T, then DMA-transpose to [t, dv] ----
                    ps_vt2 = ps.tile([128, 2, S], F32, tag="ps")
                    ps_vt = ps_vt2[:, 0, :]
                    for i in range(DT):
                        nc.tensor.matmul(ps_vt, wv_s[:, i, :], xt(i),
                                         start=(i == 0), stop=(i == DT - 1))
                    vT = work.tile([128, S], BF16, tag="vT")
                    nc.vector.tensor_copy(out=vT[:, :], in_=ps_vt)
                    for ts in range(S // 128):
                        nc.sync.dma_start(out=v_b[:, NKC + ts, :],
                                          in_=vT[:, 128 * ts:128 * ts + 128],
                                          transpose=True)

                    if it == 0 and b == 0:
                        load_wo()

                    # ---- attention, kt-outer (shared stationary per kt) ----
                    pv = [pspv.tile([128, S], F32, tag="pspv", name=f"pv{b}_{h}")
                          for h in range(NH)]
                    den = [denp.tile([128, 2, S], F32, tag=f"den{hp}",
                                     name=f"den{b}_{hp}") for hp in range(2)]
                    # final denominators, cast to bf16 incrementally as query
                    # column ranges stop receiving contributions
                    denb = [work.tile([128, 2, S], BF16, tag="denb",
                                      name=f"denb{b}_{hp}") for hp in range(2)]
                    pt_tiles = {}

                    def scores(kt):
                        vis0 = 128 * (kt - NKC) if kt >= NKC else 0
                        n = S - vis0
                        pt = ptp.tile([128, NH, S], BF16, tag="pt",
                                      name=f"pt{b}_{kt}")
                        pt_tiles[kt] = pt
                        for hp in range(2):  # head pairs share a 2-bank psum
                            ps_s = ps.tile([128, 2, S], F32, tag="ps",
                                           name=f"pss{b}_{kt}_{hp}")
                            for j in range(2):
                                nc.tensor.matmul(
                                    ps_s[:, j, 0:n],
                                    kT_b[:, 128 * kt:128 * kt + 128],
                                    qT_b[:, 2 * hp + j, vis0:S],
                                    start=True, stop=True)
                            hs = slice(2 * hp, 2 * hp + 2)
                            if kt < NKC:
                                nc.scalar.activation(
                                    pt[:, hs, :], ps_s[:, :, :],
                                    mybir.ActivationFunctionType.Exp, scale=SCALE)
                            else:
                                te = tep.tile([128, 2, 128], BF16, tag="te")
                                nc.scalar.activation(
                                    te[:, :, :], ps_s[:, :, 0:128],
                                    mybir.ActivationFunctionType.Exp, scale=SCALE)
                                nc.vector.tensor_tensor(
                                    out=pt[:, hs, vis0:vis0 + 128],
                                    in0=te[:, :, :], in1=maskd[:, :, :],
                                    op=mybir.AluOpType.mult)
                                if n > 128:
                                    nc.scalar.activation(
                                        pt[:, hs, vis0 + 128:S],
                                        ps_s[:, :, 128:n],
                                        mybir.ActivationFunctionType.Exp,
                                        scale=SCALE)
                            # denominator accumulation (in-place f32, paired)
                            if kt == 0:
                                nc.vector.tensor_copy(out=den[hp][:, :, :],
                                                      in_=pt[:, hs, :])
                            else:
                                nc.vector.tensor_tensor(
                                    out=den[hp][:, :, vis0:S],
                                    in0=den[hp][:, :, vis0:S],
                                    in1=pt[:, hs, vis0:S],
                                    op=mybir.AluOpType.add)
                            # columns [128(kt-NKC) : 128(kt-NKC+1)] final now
                            if NKC <= kt < KT - 1:
                                f0 = 128 * (kt - NKC)
                                nc.vector.tensor_copy(
                                    out=denb[hp][:, :, f0:f0 + 128],
                                    in_=den[hp][:, :, f0:f0 + 128])

                    def pv_step(kt):
                        vis0 = 128 * (kt - NKC) if kt >= NKC else 0
                        pt = pt_tiles.pop(kt)
                        for h in range(NH):
                            o = pv[h][:, :] if kt == 0 else pv[h][:, vis0:S]
                            nc.tensor.matmul(o, v_b[:, kt, :], pt[:, h, vis0:S],
                                             start=(kt == 0), stop=(kt == KT - 1))

                    SPL = 128 * (KT - 1 - NKC)  # cols final before last kt
                    psdb_t = {}

                    def finish_heads(g):
                        """Normalize heads of gather group g and launch the
                        gather. The reciprocal runs as exp(-ln(x)) on the
                        Scalar engine — ln/exp share one activation table set
                        with the softmax exps, keeping the slow DVE reciprocal
                        off the critical path. Ln stages to SBUF so the psum
                        tile frees as soon as Ln has read it."""
                        for h in range(hpg * g, hpg * g + hpg):
                            hp = h // 2
                            if hp not in psdb_t:
                                psdb_t[hp] = ps.tile([128, 2, S], F32, tag="ps",
                                                     name=f"psdb{b}_{hp}")
                            ps_db = psdb_t[hp]
                            bank = h % 2
                            nc.tensor.matmul(ps_db[:, bank, :], ones_m[:, :],
                                             denb[hp][:, bank, :],
                                             start=True, stop=True)
                            nc.scalar.activation(
                                ps_db[:, bank, :], ps_db[:, bank, :],
                                mybir.ActivationFunctionType.Ln)
                            recb = work.tile([128, S], F32, tag="recb")
                            nc.scalar.activation(
                                recb[:, :], ps_db[:, bank, :],
                                mybir.ActivationFunctionType.Exp, scale=-1.0)
                            at = atp.tile([128, S], BF16, tag="at")
                            nc.vector.tensor_tensor(
                                out=at[:, :], in0=pv[h][:, :],
                                in1=recb[:, :], op=mybir.AluOpType.mult)
                            hh = h - hpg * g
                            nc.sync.dma_start(
                                out=ag_in[b][g][128 * hh:128 * hh + 128, :],
                                in_=at[:, :])
                        for _d in range(2 if DUP_COLL else 1):
                            nc.gpsimd.collective_compute(
                                "AllGather", mybir.AluOpType.bypass,
                                replica_groups=rg,
                                ins=[ag_in[b][g][:, :]], outs=[ag_out[b][g][:, :]])

                    for kt in range(KT):
                        scores(kt)
                        if kt >= 2:
                            pv_step(kt - 2)
                    pv_step(KT - 2)
                    pv_step(KT - 1)
                    # final bf16 den slices for both head pairs, ahead of the
                    # per-head normalize chains so the den matmuls can stream
                    for hp in range(2):
                        nc.vector.tensor_copy(out=denb[hp][:, :, SPL:S],
                                              in_=den[hp][:, :, SPL:S])
                    for g in range(ngr):
                        finish_heads(g)

                    # ---- output projection, PIPE_DEPTH blocks behind ----
                    pending.append(b)
                    if len(pending) > PIPE_DEPTH:
                        emit_wo(pending.pop(0))
            for pb in pending:
                emit_wo(pb)

    _dedup_ldweights(nc)
    _split_excess_waits(nc)
    return nc


_nc_cache = {}


def _get_nc(iters: int):
    if iters not in _nc_cache:
        _nc_cache[iters] = build_nc(iters)
    return _nc_cache[iters]


def make_in_maps(x, wq, wk, wv, wo, freqs_cos, freqs_sin, cache_k, cache_v):
    bf = lambda a: np.ascontiguousarray(a).astype(NPBF16)
    xT = bf(x.reshape(NT, D).T)
    cosT = bf(freqs_cos.T)
    sinT = bf(freqs_sin.T)
    # permute rope pair dims to [evens, odds] within each head
    wq_p = wq.reshape(H, HD, D)[:, PERM, :].reshape(H * HD, D)
    wk_p = wk.reshape(HKV, HD, D)[:, PERM, :].reshape(HKV * HD, D)
    in_maps = []
    for c in range(NCORES):
        in_maps.append({
            "xT": xT,
            "wqT": bf(wq_p[EL * c:EL * (c + 1), :].T),
            "wkT": bf(wk_p[HD * c:HD * (c + 1), :].T),
            "wvT": bf(wv[HD * c:HD * (c + 1), :].T),
            "woT": bf(wo[EL * c:EL * (c + 1), :].T),
            "ckT": bf(cache_k[:, :, c, :].transpose(0, 2, 1)[:, PERM, :]),
            "cv": bf(cache_v[:, :, c, :]),
            "cosT": cosT, "sinT": sinT,
        })
    return in_maps


def assemble_out(results):
    return np.concatenate(
        [results[c]["out"].T for c in range(NCORES)], axis=1
    ).reshape(B, S, D)


def kernel(x, wq, wk, wv, wo, freqs_cos, freqs_sin, cache_k, cache_v,
           start_pos=START, **_ignored):
    assert x.shape == (B, S, D) and int(start_pos) == START
    nc = _get_nc(1)
    in_maps = make_in_maps(np.asarray(x, np.float32), np.asarray(wq, np.float32),
                           np.asarray(wk, np.float32), np.asarray(wv, np.float32),
                           np.asarray(wo, np.float32),
                           np.asarray(freqs_cos, np.float32),
                           np.asarray(freqs_sin, np.float32),
                           np.asarray(cache_k, np.float32),
                           np.asarray(cache_v, np.float32))
    res = run_bass_kernel_spmd(nc, in_maps, core_ids=list(range(NCORES)),
                               trace=False)
    return assemble_out(res.results)



# revision 29
# speedup vs baseline: 1.0379x; 1.0379x over previous
"""Tensor-parallel attention forward (B=4, S=512, D=4096, H=32, HKV=8, HD=128,
START=512) on 8 TRN2 NeuronCores.

Sharding (chosen): TP over heads. Each core c owns q-heads 4c..4c+3 (wq rows
512c:512c+512), kv-head c (wk/wv rows 128c:128c+128, cache slice c), and
output columns 512c:512c+512 (wo rows 512c:512c+512). x is replicated. After
local attention, per-core attention outputs (head-sharded) are AllGathered
(bf16, split in two per token block for earlier comm start) and each core
computes its own 512-column slice of the output projection — no reduction
collective needed. The host concatenates the 8 column slices.

Host-side layout prep (part of sharding): operands are pre-transposed so the
contraction dim (model dim d / feature dim e) lands on SBUF partitions with
natural-stride DMA, pre-cast to bf16 (the on-device compute precision — this
halves HBM traffic), and RoPE pair dims of wq/wk/cache_k are pre-permuted to
[evens, odds] so the on-chip rotation is two contiguous 64-partition blocks.

Compute: bf16 matmuls (fp32 PSUM accumulate), fp32 softmax denominators.
Causal structure: key-tile kt >= 4 only attends to queries s >= 128*(kt-4);
matmul N, exp and denominator work are trimmed accordingly, and only the
128-wide diagonal block needs the affine predicate fill.
"""
import math

import numpy as np
import ml_dtypes

import concourse.mybir as mybir
from concourse import bass
from concourse.tile import TileContext
from concourse.bass_utils import run_bass_kernel_spmd

F32 = mybir.dt.float32
BF16 = mybir.dt.bfloat16
NPBF16 = ml_dtypes.bfloat16

NCORES = 8
B, S, D = 4, 512, 4096
H, HKV, HD = 32, 8, 128
START = 512
T = START + S          # 1024 total kv length
NT = B * S             # 2048 tokens
NH = H // NCORES       # 4 local q heads
EL = NH * HD           # 512 local e width
DT = D // 128          # 32 d-tiles
KT = T // 128          # 8 k-tiles
NKC = START // 128     # 4 cached k-tiles
SCALE = 1.0 / math.sqrt(HD)

# RoPE pair permutation: head-dim reordered to [evens, odds]
PERM = np.concatenate([np.arange(0, HD, 2), np.arange(1, HD, 2)])

SPLIT_AG = True   # kept for compat; NGR is authoritative
NGR = 2           # gathers per token block (1, 2, or 4)
PIPE_DEPTH = 1    # token blocks between a gather and its output projection
PT_BUFS = 3       # probability-tile double/triple buffering
TE_BUFS = 2       # exp-staging tiles for the masked diagonal
DUP_DVE = False   # diagnostic: double rope DVE work
DUP_ACT = False   # diagnostic: double exp work
DUP_POOL = False  # diagnostic: double affine_select work
DUP_COLL = False  # diagnostic: double collectives

_counter = [0]


def _dedup_ldweights(nc):
    """Drop InstLdweights whose stationary AP is identical to the previous
    PE weight load (weights persist in the PE array across matmuls)."""
    removed = 0
    for f in nc.m.functions:
        for blk in f.blocks:
            last_sig = None
            keep = []
            for inst in blk.instructions:
                tn = type(inst).__name__
                if tn == "InstLdweights":
                    sig = (str(inst.ins[0])
                           + str(getattr(inst, "tile_position", None))
                           + str(getattr(inst, "tile_size", None)))
                    if sig == last_sig and not (inst.sync_info and inst.sync_info.on_wait):
                        removed += 1
                        continue
                    last_sig = sig
                elif tn == "InstMatmult":
                    # f32 matmuls stay self-loading (no split LDW) and
                    # clobber the PE weight array; transpose-mode matmuls
                    # change array state too
                    try:
                        if getattr(inst, "is_transpose", False) or \
                                "float32" in str(inst.ins[1].dtype):
                            last_sig = None
                    except Exception:
                        last_sig = None
                elif getattr(inst, "engine", None) == mybir.EngineType.PE:
                    last_sig = None
                keep.append(inst)
            blk.instructions = keep
    return removed


def _split_excess_waits(nc, cap: int = 1):
    """This walrus build rejects instructions with >1 sync waits; split the
    extras into leading no-ops on the same engine."""
    for f in nc.m.functions:
        for blk in f.blocks:
            insts = blk.instructions
            i = 0
            while i < len(insts):
                inst = insts[i]
                si = inst.sync_info
                if si is not None and si.on_wait is not None and len(si.on_wait) > cap:
                    waits = list(si.on_wait)
                    extra, keep = waits[:-cap], waits[-cap:]
                    nops = []
                    for j in range(0, len(extra), cap):
                        _counter[0] += 1
                        nops.append(mybir.InstNoOp(
                            name=f"waitsplit-{_counter[0]}",
                            engine=inst.engine, ins=[], outs=[],
                            sync_info=mybir.SyncInfo(
                                on_wait=extra[j:j + cap], on_update=[]),
                        ))
                    inst.sync_info = mybir.SyncInfo(
                        on_wait=keep, on_update=list(si.on_update or []))
                    for k, nop in enumerate(nops):
                        insts.insert(i + k, nop)
                    i += len(nops)
                i += 1


def build_nc(iters: int = 1):
    nc = bass.Bass(num_devices=NCORES)

    xT = nc.declare_dram_parameter("xT", [D, NT], BF16, isOutput=False)
    wqT = nc.declare_dram_parameter("wqT", [D, EL], BF16, isOutput=False)
    wkT = nc.declare_dram_parameter("wkT", [D, HD], BF16, isOutput=False)
    wvT = nc.declare_dram_parameter("wvT", [D, HD], BF16, isOutput=False)
    woT = nc.declare_dram_parameter("woT", [D, EL], BF16, isOutput=False)
    ckT = nc.declare_dram_parameter("ckT", [B, HD, START], BF16, isOutput=False)
    cv = nc.declare_dram_parameter("cv", [B, START, HD], BF16, isOutput=False)
    cosT = nc.declare_dram_parameter("cosT", [HD // 2, S], BF16, isOutput=False)
    sinT = nc.declare_dram_parameter("sinT", [HD // 2, S], BF16, isOutput=False)
    out = nc.declare_dram_parameter("out", [EL, NT], F32, isOutput=True)

    ngr = NGR
    hpg = NH // ngr  # heads per gather group
    ag_in = [[nc.dram_tensor(f"ag_in_{b}_{g}", [hpg * HD, S], BF16)
              for g in range(ngr)] for b in range(B)]
    ag_out = [[nc.dram_tensor(f"ag_out_{b}_{g}", [NCORES * hpg * HD, S], BF16,
                              addr_space="Shared") for g in range(ngr)]
              for b in range(B)]

    rg = [list(range(NCORES))]

    with TileContext(nc) as tc:
        with (
            tc.tile_pool(name="wpool", bufs=1) as wpool,
            tc.tile_pool(name="cpool", bufs=1) as cpool,
            tc.tile_pool(name="xpool", bufs=2) as xpool,
            tc.tile_pool(name="qkv", bufs=2) as qkv,
            tc.tile_pool(name="work", bufs=2) as work,
            tc.tile_pool(name="denp", bufs=1) as denp,
            tc.tile_pool(name="ptp", bufs=PT_BUFS) as ptp,
            tc.tile_pool(name="tep", bufs=TE_BUFS) as tep,
            tc.tile_pool(name="rope", bufs=2) as ropep,
            tc.tile_pool(name="atp", bufs=4) as atp,
            tc.tile_pool(name="gath", bufs=3) as gath,
            tc.tile_pool(name="ps", bufs=2, space="PSUM") as ps,
            tc.tile_pool(name="pspv", bufs=4, space="PSUM") as pspv,
        ):
            # ---- preamble (split + ordered so the first Q matmuls start early)
            wq_s = wpool.tile([128, DT, EL], BF16, tag="wq")
            for q in range(4):
                nc.scalar.dma_start(
                    out=wq_s[:, 8 * q:8 * q + 8, :],
                    in_=wqT[:, :].rearrange("(i p) e -> p i e", p=128)[:, 8 * q:8 * q + 8, :])
            wk_s = wpool.tile([128, DT, HD], BF16, tag="wk")
            nc.scalar.dma_start(out=wk_s[:, :, :],
                                in_=wkT[:, :].rearrange("(i p) e -> p i e", p=128))
            wv_s = wpool.tile([128, DT, HD], BF16, tag="wv")
            nc.scalar.dma_start(out=wv_s[:, :, :],
                                in_=wvT[:, :].rearrange("(i p) e -> p i e", p=128))
            cos_s = cpool.tile([64, S], BF16, tag="cos")
            nc.scalar.dma_start(out=cos_s[:, :], in_=cosT[:, :])
            sin_s = cpool.tile([64, S], BF16, tag="sin")
            nc.scalar.dma_start(out=sin_s[:, :], in_=sinT[:, :])
            ones_m = cpool.tile([128, 128], BF16, tag="onm")
            nc.vector.memset(ones_m[:, :], 1.0)
            # causal mask for the diagonal key tile: maskd[p, :, j] = (j >= p)
            maskd = cpool.tile([128, 2, 128], BF16, tag="maskd")
            nc.vector.memset(maskd[:, :, :], 1.0)
            nc.gpsimd.affine_select(
                out=maskd[:, :, :], in_=maskd[:, :, :],
                pattern=[[0, 2], [1, 128]],
                compare_op=mybir.AluOpType.is_ge,
                fill=0.0, base=0, channel_multiplier=-1)
            # wo is not needed until the first output projection — load late
            wo_s = wpool.tile([128, DT, EL], BF16, tag="wo")

            def load_wo():
                for q in range(4):
                    nc.scalar.dma_start(
                        out=wo_s[:, 8 * q:8 * q + 8, :],
                        in_=woT[:, :].rearrange("(i p) e -> p i e", p=128)[:, 8 * q:8 * q + 8, :])

            def rope(dst_a, dst_b, src):
                """dst = rotate(src); src [128, S] PSUM f32 with partitions
                [evens(a) 0:64, odds(b) 64:128]; dst bf16 [64, S] slices."""
                for _dup in range(2 if DUP_DVE else 1):
                    _rope1(dst_a, dst_b, src)

            def _rope1(dst_a, dst_b, src):
                a, bb = src[0:64, :], src[64:128, :]
                t1 = ropep.tile([64, S], BF16, tag="rt1")
                t2 = ropep.tile([64, S], BF16, tag="rt2")
                nc.vector.tensor_tensor(out=t1[:, :], in0=a, in1=cos_s[:, :],
                                        op=mybir.AluOpType.mult)
                nc.vector.tensor_tensor(out=t2[:, :], in0=bb, in1=sin_s[:, :],
                                        op=mybir.AluOpType.mult)
                nc.vector.tensor_tensor(out=dst_a, in0=t1[:, :], in1=t2[:, :],
                                        op=mybir.AluOpType.subtract)
                t3 = ropep.tile([64, S], BF16, tag="rt3")
                t4 = ropep.tile([64, S], BF16, tag="rt4")
                nc.vector.tensor_tensor(out=t3[:, :], in0=a, in1=sin_s[:, :],
                                        op=mybir.AluOpType.mult)
                nc.vector.tensor_tensor(out=t4[:, :], in0=bb, in1=cos_s[:, :],
                                        op=mybir.AluOpType.mult)
                nc.vector.tensor_tensor(out=dst_b, in0=t3[:, :], in1=t4[:, :],
                                        op=mybir.AluOpType.add)

            def emit_wo(b):
                """Output projection for block b from the gathers. Gather g's
                tile index i covers e-tile 4*(i//hpg) + hpg*g + i%hpg. The
                gathered activations are read back as half-gather 1MB HWDGE
                DMAs on the sync ring (prefetchable during the next block)."""
                ps_y = [pspv.tile([128, S], F32, tag="pspv", name=f"psy{b}_{dj}")
                        for dj in range(4)]
                nchunk = NCORES * hpg  # tiles per gather
                for g in range(ngr):
                    src = ag_out[b][g][:, :].rearrange("(i p) q -> p i q", p=128)
                    for half in range(2):
                        i0 = (nchunk // 2) * half
                        ag_t = gath.tile([128, nchunk // 2, S], BF16, tag="agt")
                        nc.sync.dma_start(
                            out=ag_t[:, :, :],
                            in_=src[:, i0:i0 + nchunk // 2, :])
                        # dj-outer: the first chunk's matmuls touch only
                        # psy[0], so the projection can begin as soon as the
                        # first pv bank is normalized and released
                        for dj in range(4):
                            for i2 in range(nchunk // 2):
                                i = i0 + i2
                                c, t2 = divmod(i, hpg)
                                e = 4 * c + hpg * g + t2
                                nc.tensor.matmul(
                                    ps_y[dj][:, :],
                                    wo_s[:, e, 128 * dj:128 * dj + 128],
                                    ag_t[:, i2, :],
                                    start=(g == 0 and half == 0 and i2 == 0),
                                    stop=(g == ngr - 1 and half == 1
                                          and i2 == nchunk // 2 - 1))
                for dj in range(4):
                    yt = work.tile([128, S], F32, tag="yt")
                    nc.vector.tensor_copy(out=yt[:, :], in_=ps_y[dj][:, :])
                    nc.gpsimd.dma_start(
                        out=out[128 * dj:128 * dj + 128, S * b:S * b + S],
                        in_=yt[:, :])

            pending = []
            for it in range(iters):
                for b in range(B):
                    # ---- loads for this token block (= batch b) ----
                    xt0 = xpool.tile([128, DT // 2, S], BF16, tag="xt")
                    xt1 = xpool.tile([128, DT // 2, S], BF16, tag="xt")
                    xsrc = xT[:, S * b:S * b + S].rearrange("(i p) t -> p i t", p=128)
                    for hh in range(2):
                        nc.gpsimd.dma_start(out=xt0[:, 8 * hh:8 * hh + 8, :],
                                            in_=xsrc[:, 8 * hh:8 * hh + 8, :])
                    for hh in range(2):
                        nc.gpsimd.dma_start(out=xt1[:, 8 * hh:8 * hh + 8, :],
                                            in_=xsrc[:, 16 + 8 * hh:16 + 8 * hh + 8, :])

                    def xt(i):
                        return (xt0 if i < DT // 2 else xt1)[:, i % (DT // 2), :]

                    kT_b = qkv.tile([128, T], BF16, tag="kT")
                    nc.sync.dma_start(out=kT_b[:, 0:START], in_=ckT[b])
                    v_b = qkv.tile([128, KT, HD], BF16, tag="v")
                    nc.sync.dma_start(
                        out=v_b[:, 0:NKC, :],
                        in_=cv[b].rearrange("(kt p) dv -> p kt dv", p=128))
                    qT_b = qkv.tile([128, NH, S], BF16, tag="qT")

                    # ---- Q projection + rope (per local head) ----
                    for j in range(NH):
                        ps_q2 = ps.tile([128, 2, S], F32, tag="ps", name=f"psq{b}_{j}")
                        ps_q = ps_q2[:, 0, :]
                        for i in range(DT):
                            nc.tensor.matmul(
                                ps_q, wq_s[:, i, 128 * j:128 * j + 128],
                                xt(i), start=(i == 0), stop=(i == DT - 1))
                        rope(qT_b[0:64, j, :], qT_b[64:128, j, :], ps_q)

                    # ---- K projection + rope ----
                    ps_k2 = ps.tile([128, 2, S], F32, tag="ps")
                    ps_k = ps_k2[:, 0, :]
                    for i in range(DT):
                        nc.tensor.matmul(ps_k, wk_s[:, i, :], xt(i),
                                         start=(i == 0), stop=(i == DT - 1))
                    rope(kT_b[0:64, START:T], kT_b[64:128, START:T], ps_k)

                    # ---- V projection as V^T, then DMA-transpose to [t, dv] ----
                    ps_vt2 = ps.tile([128, 2, S], F32, tag="ps")
                    ps_vt = ps_vt2[:, 0, :]
                    for i in range(DT):
                        nc.tensor.matmul(ps_vt, wv_s[:, i, :], xt(i),
                                         start=(i == 0), stop=(i == DT - 1))
                    vT = work.tile([128, S], BF16, tag="vT")
                    nc.vector.tensor_copy(out=vT[:, :], in_=ps_vt)
                    for ts in range(S // 128):
                        nc.sync.dma_start(out=v_b[:, NKC + ts, :],
                                          in_=vT[:, 128 * ts:128 * ts + 128],
                                          transpose=True)

                    if it == 0 and b == 0:
                        load_wo()

                    # ---- attention, kt-outer (shared stationary per kt) ----
                    pv = [pspv.tile([128, S], F32, tag="pspv", name=f"pv{b}_{h}")
                          for h in range(NH)]
                    den = [denp.tile([128, 2, S], F32, tag=f"den{hp}",
                                     name=f"den{b}_{hp}") for hp in range(2)]
                    # final denominators, cast to bf16 incrementally as query
                    # column ranges stop receiving contributions
                    denb = [work.tile([128, 2, S], BF16, tag="denb",
                                      name=f"denb{b}_{hp}") for hp in range(2)]
                    pt_tiles = {}

                    def scores(kt):
                        vis0 = 128 * (kt - NKC) if kt >= NKC else 0
                        n = S - vis0
                        pt = ptp.tile([128, NH, S], BF16, tag="pt",
                                      name=f"pt{b}_{kt}")
                        pt_tiles[kt] = pt
                        for hp in range(2):  # head pairs share a 2-bank psum
                            ps_s = ps.tile([128, 2, S], F32, tag="ps",
                                           name=f"pss{b}_{kt}_{hp}")
                            for j in range(2):
                                nc.tensor.matmul(
                                    ps_s[:, j, 0:n],
                                    kT_b[:, 128 * kt:128 * kt + 128],
                                    qT_b[:, 2 * hp + j, vis0:S],
                                    start=True, stop=True)
                            hs = slice(2 * hp, 2 * hp + 2)
                            if kt < NKC:
                                nc.scalar.activation(
                                    pt[:, hs, :], ps_s[:, :, :],
                                    mybir.ActivationFunctionType.Exp, scale=SCALE)
                            else:
                                te = tep.tile([128, 2, 128], BF16, tag="te")
                                nc.scalar.activation(
                                    te[:, :, :], ps_s[:, :, 0:128],
                                    mybir.ActivationFunctionType.Exp, scale=SCALE)
                                nc.vector.tensor_tensor(
                                    out=pt[:, hs, vis0:vis0 + 128],
                                    in0=te[:, :, :], in1=maskd[:, :, :],
                                    op=mybir.AluOpType.mult)
                                if n > 128:
                                    nc.scalar.activation(
                                        pt[:, hs, vis0 + 128:S],
                                        ps_s[:, :, 128:n],
                                        mybir.ActivationFunctionType.Exp,
                                        scale=SCALE)
                            # denominator accumulation (in-place f32, paired)
                            if kt == 0:
                                nc.vector.tensor_copy(out=den[hp][:, :, :],
                                                      in_=pt[:, hs, :])
                            else:
                                nc.vector.tensor_tensor(
                                    out=den[hp][:, :, vis0:S],
                                    in0=den[hp][:, :, vis0:S],
                                    in1=pt[:, hs, vis0:S],
                                    op=mybir.AluOpType.add)
                            # columns [128(kt-NKC) : 128(kt-NKC+1)] final now
                            if NKC <= kt < KT - 1:
                                f0 = 128 * (kt - NKC)
                                nc.vector.tensor_copy(
                                    out=denb[hp][:, :, f0:f0 + 128],
                                    in_=den[hp][:, :, f0:f0 + 128])

                    def pv_step(kt):
                        vis0 = 128 * (kt - NKC) if kt >= NKC else 0
                        pt = pt_tiles.pop(kt)
                        for h in range(NH):
                            o = pv[h][:, :] if kt == 0 else pv[h][:, vis0:S]
                            nc.tensor.matmul(o, v_b[:, kt, :], pt[:, h, vis0:S],
                                             start=(kt == 0), stop=(kt == KT - 1))

                    SPL = 128 * (KT - 1 - NKC)  # cols final before last kt
                    psdb_t = {}

                    rec_t = {}

                    def recip_head(h):
                        """recb[h] = 1/sum_keys(exp) via broadcast-matmul +
                        exp(-ln(x)) on the Scalar engine — ln/exp share one
                        activation table set with the softmax exps, keeping
                        the slow DVE reciprocal off the critical path. Head
                        pairs share one 2-bank PSUM tile (ln in place)."""
                        hp = h // 2
                        if hp not in psdb_t:
                            psdb_t[hp] = ps.tile([128, 2, S], F32, tag="ps",
                                                 name=f"psdb{b}_{hp}")
                        ps_db = psdb_t[hp]
                        bank = h % 2
                        nc.tensor.matmul(ps_db[:, bank, :], ones_m[:, :],
                                         denb[hp][:, bank, :],
                                         start=True, stop=True)
                        nc.scalar.activation(
                            ps_db[:, bank, :], ps_db[:, bank, :],
                            mybir.ActivationFunctionType.Ln)
                        recb = atp.tile([128, S], F32, tag="recb",
                                         name=f"recb{b}_{h}")
                        nc.scalar.activation(
                            recb[:, :], ps_db[:, bank, :],
                            mybir.ActivationFunctionType.Exp, scale=-1.0)
                        rec_t[h] = recb

                    def finish_g(g):
                        for h in range(hpg * g, hpg * g + hpg):
                            at = atp.tile([128, S], BF16, tag="at")
                            nc.vector.tensor_tensor(
                                out=at[:, :], in0=pv[h][:, :],
                                in1=rec_t.pop(h)[:, :],
                                op=mybir.AluOpType.mult)
                            hh = h - hpg * g
                            nc.sync.dma_start(
                                out=ag_in[b][g][128 * hh:128 * hh + 128, :],
                                in_=at[:, :])
                        for _d in range(2 if DUP_COLL else 1):
                            nc.gpsimd.collective_compute(
                                "AllGather", mybir.AluOpType.bypass,
                                replica_groups=rg,
                                ins=[ag_in[b][g][:, :]], outs=[ag_out[b][g][:, :]])

                    for kt in range(KT):
                        scores(kt)
                        if kt >= 2:
                            pv_step(kt - 2)
                    pv_step(KT - 2)
                    pv_step(KT - 1)
                    # final bf16 den slices for both head pairs, ahead of the
                    # per-head normalize chains so the den matmuls can stream
                    for hp in range(2):
                        nc.vector.tensor_copy(out=denb[hp][:, :, SPL:S],
                                              in_=den[hp][:, :, SPL:S])
                    # interleave head pairs so consecutive den matmuls hit
                    # different PSUM tiles (no write-after-read stall)
                    for h in ((0, 2, 1, 3) if NH == 4 else range(NH)):
                        recip_head(h)
                    for g in range(ngr):
                        finish_g(g)

                    # ---- output projection, PIPE_DEPTH blocks behind ----
                    pending.append(b)
                    if len(pending) > PIPE_DEPTH:
                        emit_wo(pending.pop(0))
            for pb in pending:
                emit_wo(pb)

    _dedup_ldweights(nc)
    _split_excess_waits(nc)
    return nc


_nc_cache = {}


def _get_nc(iters: int):
    if iters not in _nc_cache:
        _nc_cache[iters] = build_nc(iters)
    return _nc_cache[iters]


def make_in_maps(x, wq, wk, wv, wo, freqs_cos, freqs_sin, cache_k, cache_v):
    bf = lambda a: np.ascontiguousarray(a).astype(NPBF16)
    xT = bf(x.reshape(NT, D).T)
    cosT = bf(freqs_cos.T)
    sinT = bf(freqs_sin.T)
    # permute rope pair dims to [evens, odds] within each head
    wq_p = wq.reshape(H, HD, D)[:, PERM, :].reshape(H * HD, D)
    wk_p = wk.reshape(HKV, HD, D)[:, PERM, :].reshape(HKV * HD, D)
    in_maps = []
    for c in range(NCORES):
        in_maps.append({
            "xT": xT,
            "wqT": bf(wq_p[EL * c:EL * (c + 1), :].T),
            "wkT": bf(wk_p[HD * c:HD * (c + 1), :].T),
            "wvT": bf(wv[HD * c:HD * (c + 1), :].T),
            "woT": bf(wo[EL * c:EL * (c + 1), :].T),
            "ckT": bf(cache_k[:, :, c, :].transpose(0, 2, 1)[:, PERM, :]),
            "cv": bf(cache_v[:, :, c, :]),
            "cosT": cosT, "sinT": sinT,
        })
    return in_maps


def assemble_out(results):
    return np.concatenate(
        [results[c]["out"].T for c in range(NCORES)], axis=1
    ).reshape(B, S, D)


def kernel(x, wq, wk, wv, wo, freqs_cos, freqs_sin, cache_k, cache_v,
           start_pos=START, **_ignored):
    assert x.shape == (B, S, D) and int(start_pos) == START
    nc = _get_nc(1)
    in_maps = make_in_maps(np.asarray(x, np.float32), np.asarray(wq, np.float32),
                           np.asarray(wk, np.float32), np.asarray(wv, np.float32),
                           np.asarray(wo, np.float32),
                           np.asarray(freqs_cos, np.float32),
                           np.asarray(freqs_sin, np.float32),
                           np.asarray(cache_k, np.float32),
                           np.asarray(cache_v, np.float32))
    res = run_bass_kernel_spmd(nc, in_maps, core_ids=list(range(NCORES)),
                               trace=False)
    return assemble_out(res.results)



# revision 30
# speedup vs baseline: 1.1264x; 1.0853x over previous
"""Tensor-parallel attention forward (B=4, S=512, D=4096, H=32, HKV=8, HD=128,
START=512) on 8 TRN2 NeuronCores.

Sharding (chosen): TP over heads. Each core c owns q-heads 4c..4c+3 (wq rows
512c:512c+512), kv-head c (wk/wv rows 128c:128c+128, cache slice c), and
output columns 512c:512c+512 (wo rows 512c:512c+512). x is replicated. After
local attention, per-core attention outputs (head-sharded) are AllGathered
(bf16, split in two per token block for earlier comm start) and each core
computes its own 512-column slice of the output projection — no reduction
collective needed. The host concatenates the 8 column slices.

Host-side layout prep (part of sharding): operands are pre-transposed so the
contraction dim (model dim d / feature dim e) lands on SBUF partitions with
natural-stride DMA, pre-cast to bf16 (the on-device compute precision — this
halves HBM traffic), and RoPE pair dims of wq/wk/cache_k are pre-permuted to
[evens, odds] so the on-chip rotation is two contiguous 64-partition blocks.

Compute: bf16 matmuls (fp32 PSUM accumulate), fp32 softmax denominator
accumulation (cast to bf16 incrementally as query column ranges finalize).
Causal structure: key-tile kt >= 4 only attends to queries s >= 128*(kt-4);
matmul N, exp and denominator work are trimmed accordingly; the 128-wide
diagonal block is masked by a DVE multiply with a precomputed 0/1 tile
(keeps the GpSimd queue off the critical path).

Engine/queue assignment (chosen to keep the softmax-critical Scalar and
Vector FIFOs free of DMA head-of-line blocking):
  - Scalar (ACT) ring: weight preamble only; ACT compute = exps + the
    softmax reciprocal as exp(-ln(x)) (ln/exp live in one activation table
    set, so there is no table ping-pong and the slow DVE reciprocal is
    avoided entirely).
  - Sync (SP) ring: kv-cache loads, V DMA-transposes, gather readbacks
    (two 1MB HWDGE loads per gather, prefetched during the next block),
    ag_in stores.
  - GpSimd (SWDGE): x loads, final output stores.
The block tail is latency-trimmed: den matmuls interleave across the two
psum den tiles, and the output projection consumes gathered tiles dj-outer
so its first 8 matmuls only need the first pv bank to be released.
"""
import math

import numpy as np
import ml_dtypes

import concourse.mybir as mybir
from concourse import bass
from concourse.tile import TileContext
from concourse.bass_utils import run_bass_kernel_spmd

F32 = mybir.dt.float32
BF16 = mybir.dt.bfloat16
NPBF16 = ml_dtypes.bfloat16

NCORES = 8
B, S, D = 4, 512, 4096
H, HKV, HD = 32, 8, 128
START = 512
T = START + S          # 1024 total kv length
NT = B * S             # 2048 tokens
NH = H // NCORES       # 4 local q heads
EL = NH * HD           # 512 local e width
DT = D // 128          # 32 d-tiles
KT = T // 128          # 8 k-tiles
NKC = START // 128     # 4 cached k-tiles
SCALE = 1.0 / math.sqrt(HD)

# RoPE pair permutation: head-dim reordered to [evens, odds]
PERM = np.concatenate([np.arange(0, HD, 2), np.arange(1, HD, 2)])

SPLIT_AG = True   # kept for compat; NGR is authoritative
NGR = 2           # gathers per token block (1, 2, or 4)
PIPE_DEPTH = 1    # token blocks between a gather and its output projection
PT_BUFS = 3       # probability-tile double/triple buffering
TE_BUFS = 2       # exp-staging tiles for the masked diagonal
DUP_DVE = False   # diagnostic: double rope DVE work
DUP_ACT = False   # diagnostic: double exp work
DUP_POOL = False  # diagnostic: double affine_select work
DUP_COLL = False  # diagnostic: double collectives

_counter = [0]


def _dedup_ldweights(nc):
    """Drop InstLdweights whose stationary AP is identical to the previous
    PE weight load (weights persist in the PE array across matmuls)."""
    removed = 0
    for f in nc.m.functions:
        for blk in f.blocks:
            last_sig = None
            keep = []
            for inst in blk.instructions:
                tn = type(inst).__name__
                if tn == "InstLdweights":
                    sig = (str(inst.ins[0])
                           + str(getattr(inst, "tile_position", None))
                           + str(getattr(inst, "tile_size", None)))
                    if sig == last_sig and not (inst.sync_info and inst.sync_info.on_wait):
                        removed += 1
                        continue
                    last_sig = sig
                elif tn == "InstMatmult":
                    # f32 matmuls stay self-loading (no split LDW) and
                    # clobber the PE weight array; transpose-mode matmuls
                    # change array state too
                    try:
                        if getattr(inst, "is_transpose", False) or \
                                "float32" in str(inst.ins[1].dtype):
                            last_sig = None
                    except Exception:
                        last_sig = None
                elif getattr(inst, "engine", None) == mybir.EngineType.PE:
                    last_sig = None
                keep.append(inst)
            blk.instructions = keep
    return removed


def _split_excess_waits(nc, cap: int = 1):
    """This walrus build rejects instructions with >1 sync waits; split the
    extras into leading no-ops on the same engine."""
    for f in nc.m.functions:
        for blk in f.blocks:
            insts = blk.instructions
            i = 0
            while i < len(insts):
                inst = insts[i]
                si = inst.sync_info
                if si is not None and si.on_wait is not None and len(si.on_wait) > cap:
                    waits = list(si.on_wait)
                    extra, keep = waits[:-cap], waits[-cap:]
                    nops = []
                    for j in range(0, len(extra), cap):
                        _counter[0] += 1
                        nops.append(mybir.InstNoOp(
                            name=f"waitsplit-{_counter[0]}",
                            engine=inst.engine, ins=[], outs=[],
                            sync_info=mybir.SyncInfo(
                                on_wait=extra[j:j + cap], on_update=[]),
                        ))
                    inst.sync_info = mybir.SyncInfo(
                        on_wait=keep, on_update=list(si.on_update or []))
                    for k, nop in enumerate(nops):
                        insts.insert(i + k, nop)
                    i += len(nops)
                i += 1


def build_nc(iters: int = 1):
    nc = bass.Bass(num_devices=NCORES)

    xT = nc.declare_dram_parameter("xT", [D, NT], BF16, isOutput=False)
    wqT = nc.declare_dram_parameter("wqT", [D, EL], BF16, isOutput=False)
    wkT = nc.declare_dram_parameter("wkT", [D, HD], BF16, isOutput=False)
    wvT = nc.declare_dram_parameter("wvT", [D, HD], BF16, isOutput=False)
    woT = nc.declare_dram_parameter("woT", [D, EL], BF16, isOutput=False)
    ckT = nc.declare_dram_parameter("ckT", [B, HD, START], BF16, isOutput=False)
    cv = nc.declare_dram_parameter("cv", [B, START, HD], BF16, isOutput=False)
    cosT = nc.declare_dram_parameter("cosT", [HD // 2, S], BF16, isOutput=False)
    sinT = nc.declare_dram_parameter("sinT", [HD // 2, S], BF16, isOutput=False)
    out = nc.declare_dram_parameter("out", [EL, NT], F32, isOutput=True)

    ngr = NGR
    hpg = NH // ngr  # heads per gather group
    ag_in = [[nc.dram_tensor(f"ag_in_{b}_{g}", [hpg * HD, S], BF16)
              for g in range(ngr)] for b in range(B)]
    ag_out = [[nc.dram_tensor(f"ag_out_{b}_{g}", [NCORES * hpg * HD, S], BF16,
                              addr_space="Shared") for g in range(ngr)]
              for b in range(B)]

    rg = [list(range(NCORES))]

    with TileContext(nc) as tc:
        with (
            tc.tile_pool(name="wpool", bufs=1) as wpool,
            tc.tile_pool(name="cpool", bufs=1) as cpool,
            tc.tile_pool(name="xpool", bufs=2) as xpool,
            tc.tile_pool(name="qkv", bufs=2) as qkv,
            tc.tile_pool(name="work", bufs=2) as work,
            tc.tile_pool(name="denp", bufs=1) as denp,
            tc.tile_pool(name="ptp", bufs=PT_BUFS) as ptp,
            tc.tile_pool(name="tep", bufs=TE_BUFS) as tep,
            tc.tile_pool(name="rope", bufs=2) as ropep,
            tc.tile_pool(name="atp", bufs=4) as atp,
            tc.tile_pool(name="gath", bufs=3) as gath,
            tc.tile_pool(name="ps", bufs=2, space="PSUM") as ps,
            tc.tile_pool(name="pspv", bufs=4, space="PSUM") as pspv,
        ):
            # ---- preamble (split + ordered so the first Q matmuls start early)
            wq_s = wpool.tile([128, DT, EL], BF16, tag="wq")
            for q in range(4):
                nc.scalar.dma_start(
                    out=wq_s[:, 8 * q:8 * q + 8, :],
                    in_=wqT[:, :].rearrange("(i p) e -> p i e", p=128)[:, 8 * q:8 * q + 8, :])
            wk_s = wpool.tile([128, DT, HD], BF16, tag="wk")
            nc.scalar.dma_start(out=wk_s[:, :, :],
                                in_=wkT[:, :].rearrange("(i p) e -> p i e", p=128))
            wv_s = wpool.tile([128, DT, HD], BF16, tag="wv")
            nc.scalar.dma_start(out=wv_s[:, :, :],
                                in_=wvT[:, :].rearrange("(i p) e -> p i e", p=128))
            cos_s = cpool.tile([64, S], BF16, tag="cos")
            nc.scalar.dma_start(out=cos_s[:, :], in_=cosT[:, :])
            sin_s = cpool.tile([64, S], BF16, tag="sin")
            nc.scalar.dma_start(out=sin_s[:, :], in_=sinT[:, :])
            ones_m = cpool.tile([128, 128], BF16, tag="onm")
            nc.vector.memset(ones_m[:, :], 1.0)
            # causal mask for the diagonal key tile: maskd[p, :, j] = (j >= p)
            maskd = cpool.tile([128, 2, 128], BF16, tag="maskd")
            nc.vector.memset(maskd[:, :, :], 1.0)
            nc.gpsimd.affine_select(
                out=maskd[:, :, :], in_=maskd[:, :, :],
                pattern=[[0, 2], [1, 128]],
                compare_op=mybir.AluOpType.is_ge,
                fill=0.0, base=0, channel_multiplier=-1)
            # wo is not needed until the first output projection — load late
            wo_s = wpool.tile([128, DT, EL], BF16, tag="wo")

            def load_wo():
                for q in range(4):
                    nc.scalar.dma_start(
                        out=wo_s[:, 8 * q:8 * q + 8, :],
                        in_=woT[:, :].rearrange("(i p) e -> p i e", p=128)[:, 8 * q:8 * q + 8, :])

            def rope(dst_a, dst_b, src):
                """dst = rotate(src); src [128, S] PSUM f32 with partitions
                [evens(a) 0:64, odds(b) 64:128]; dst bf16 [64, S] slices."""
                for _dup in range(2 if DUP_DVE else 1):
                    _rope1(dst_a, dst_b, src)

            def _rope1(dst_a, dst_b, src):
                a, bb = src[0:64, :], src[64:128, :]
                t1 = ropep.tile([64, S], BF16, tag="rt1")
                t2 = ropep.tile([64, S], BF16, tag="rt2")
                nc.vector.tensor_tensor(out=t1[:, :], in0=a, in1=cos_s[:, :],
                                        op=mybir.AluOpType.mult)
                nc.vector.tensor_tensor(out=t2[:, :], in0=bb, in1=sin_s[:, :],
                                        op=mybir.AluOpType.mult)
                nc.vector.tensor_tensor(out=dst_a, in0=t1[:, :], in1=t2[:, :],
                                        op=mybir.AluOpType.subtract)
                t3 = ropep.tile([64, S], BF16, tag="rt3")
                t4 = ropep.tile([64, S], BF16, tag="rt4")
                nc.vector.tensor_tensor(out=t3[:, :], in0=a, in1=sin_s[:, :],
                                        op=mybir.AluOpType.mult)
                nc.vector.tensor_tensor(out=t4[:, :], in0=bb, in1=cos_s[:, :],
                                        op=mybir.AluOpType.mult)
                nc.vector.tensor_tensor(out=dst_b, in0=t3[:, :], in1=t4[:, :],
                                        op=mybir.AluOpType.add)

            def emit_wo(b):
                """Output projection for block b from the gathers. Gather g's
                tile index i covers e-tile 4*(i//hpg) + hpg*g + i%hpg. The
                gathered activations are read back as half-gather 1MB HWDGE
                DMAs on the sync ring (prefetchable during the next block)."""
                ps_y = [pspv.tile([128, S], F32, tag="pspv", name=f"psy{b}_{dj}")
                        for dj in range(4)]
                nchunk = NCORES * hpg  # tiles per gather
                for g in range(ngr):
                    src = ag_out[b][g][:, :].rearrange("(i p) q -> p i q", p=128)
                    for half in range(2):
                        i0 = (nchunk // 2) * half
                        ag_t = gath.tile([128, nchunk // 2, S], BF16, tag="agt")
                        nc.sync.dma_start(
                            out=ag_t[:, :, :],
                            in_=src[:, i0:i0 + nchunk // 2, :])
                        # dj-outer: the first chunk's matmuls touch only
                        # psy[0], so the projection can begin as soon as the
                        # first pv bank is normalized and released
                        for dj in range(4):
                            for i2 in range(nchunk // 2):
                                i = i0 + i2
                                c, t2 = divmod(i, hpg)
                                e = 4 * c + hpg * g + t2
                                nc.tensor.matmul(
                                    ps_y[dj][:, :],
                                    wo_s[:, e, 128 * dj:128 * dj + 128],
                                    ag_t[:, i2, :],
                                    start=(g == 0 and half == 0 and i2 == 0),
                                    stop=(g == ngr - 1 and half == 1
                                          and i2 == nchunk // 2 - 1))
                for dj in range(4):
                    yt = work.tile([128, S], F32, tag="yt")
                    nc.vector.tensor_copy(out=yt[:, :], in_=ps_y[dj][:, :])
                    nc.gpsimd.dma_start(
                        out=out[128 * dj:128 * dj + 128, S * b:S * b + S],
                        in_=yt[:, :])

            pending = []
            for it in range(iters):
                for b in range(B):
                    # ---- loads for this token block (= batch b) ----
                    xt0 = xpool.tile([128, DT // 2, S], BF16, tag="xt")
                    xt1 = xpool.tile([128, DT // 2, S], BF16, tag="xt")
                    xsrc = xT[:, S * b:S * b + S].rearrange("(i p) t -> p i t", p=128)
                    for hh in range(2):
                        nc.gpsimd.dma_start(out=xt0[:, 8 * hh:8 * hh + 8, :],
                                            in_=xsrc[:, 8 * hh:8 * hh + 8, :])
                    for hh in range(2):
                        nc.gpsimd.dma_start(out=xt1[:, 8 * hh:8 * hh + 8, :],
                                            in_=xsrc[:, 16 + 8 * hh:16 + 8 * hh + 8, :])

                    def xt(i):
                        return (xt0 if i < DT // 2 else xt1)[:, i % (DT // 2), :]

                    kT_b = qkv.tile([128, T], BF16, tag="kT")
                    nc.sync.dma_start(out=kT_b[:, 0:START], in_=ckT[b])
                    v_b = qkv.tile([128, KT, HD], BF16, tag="v")
                    nc.sync.dma_start(
                        out=v_b[:, 0:NKC, :],
                        in_=cv[b].rearrange("(kt p) dv -> p kt dv", p=128))
                    qT_b = qkv.tile([128, NH, S], BF16, tag="qT")

                    # ---- Q projection + rope (per local head) ----
                    for j in range(NH):
                        ps_q2 = ps.tile([128, 2, S], F32, tag="ps", name=f"psq{b}_{j}")
                        ps_q = ps_q2[:, 0, :]
                        for i in range(DT):
                            nc.tensor.matmul(
                                ps_q, wq_s[:, i, 128 * j:128 * j + 128],
                                xt(i), start=(i == 0), stop=(i == DT - 1))
                        rope(qT_b[0:64, j, :], qT_b[64:128, j, :], ps_q)

                    # ---- K projection + rope ----
                    ps_k2 = ps.tile([128, 2, S], F32, tag="ps")
                    ps_k = ps_k2[:, 0, :]
                    for i in range(DT):
                        nc.tensor.matmul(ps_k, wk_s[:, i, :], xt(i),
                                         start=(i == 0), stop=(i == DT - 1))
                    rope(kT_b[0:64, START:T], kT_b[64:128, START:T], ps_k)

                    # ---- V projection as V^T, then DMA-transpose to [t, dv] ----
                    ps_vt2 = ps.tile([128, 2, S], F32, tag="ps")
                    ps_vt = ps_vt2[:, 0, :]
                    for i in range(DT):
                        nc.tensor.matmul(ps_vt, wv_s[:, i, :], xt(i),
                                         start=(i == 0), stop=(i == DT - 1))
                    vT = work.tile([128, S], BF16, tag="vT")
                    nc.vector.tensor_copy(out=vT[:, :], in_=ps_vt)
                    for ts in range(S // 128):
                        nc.sync.dma_start(out=v_b[:, NKC + ts, :],
                                          in_=vT[:, 128 * ts:128 * ts + 128],
                                          transpose=True)

                    if it == 0 and b == 0:
                        load_wo()

                    # ---- attention, kt-outer (shared stationary per kt) ----
                    pv = [pspv.tile([128, S], F32, tag="pspv", name=f"pv{b}_{h}")
                          for h in range(NH)]
                    den = [denp.tile([128, 2, S], F32, tag=f"den{hp}",
                                     name=f"den{b}_{hp}") for hp in range(2)]
                    # final denominators, cast to bf16 incrementally as query
                    # column ranges stop receiving contributions
                    denb = [work.tile([128, 2, S], BF16, tag="denb",
                                      name=f"denb{b}_{hp}") for hp in range(2)]
                    pt_tiles = {}

                    def scores(kt):
                        vis0 = 128 * (kt - NKC) if kt >= NKC else 0
                        n = S - vis0
                        pt = ptp.tile([128, NH, S], BF16, tag="pt",
                                      name=f"pt{b}_{kt}")
                        pt_tiles[kt] = pt
                        for hp in range(2):  # head pairs share a 2-bank psum
                            ps_s = ps.tile([128, 2, S], F32, tag="ps",
                                           name=f"pss{b}_{kt}_{hp}")
                            for j in range(2):
                                nc.tensor.matmul(
                                    ps_s[:, j, 0:n],
                                    kT_b[:, 128 * kt:128 * kt + 128],
                                    qT_b[:, 2 * hp + j, vis0:S],
                                    start=True, stop=True)
                            hs = slice(2 * hp, 2 * hp + 2)
                            if kt < NKC:
                                nc.scalar.activation(
                                    pt[:, hs, :], ps_s[:, :, :],
                                    mybir.ActivationFunctionType.Exp, scale=SCALE)
                            else:
                                te = tep.tile([128, 2, 128], BF16, tag="te")
                                nc.scalar.activation(
                                    te[:, :, :], ps_s[:, :, 0:128],
                                    mybir.ActivationFunctionType.Exp, scale=SCALE)
                                nc.vector.tensor_tensor(
                                    out=pt[:, hs, vis0:vis0 + 128],
                                    in0=te[:, :, :], in1=maskd[:, :, :],
                                    op=mybir.AluOpType.mult)
                                if n > 128:
                                    nc.scalar.activation(
                                        pt[:, hs, vis0 + 128:S],
                                        ps_s[:, :, 128:n],
                                        mybir.ActivationFunctionType.Exp,
                                        scale=SCALE)
                            # denominator accumulation (in-place f32, paired)
                            if kt == 0:
                                nc.vector.tensor_copy(out=den[hp][:, :, :],
                                                      in_=pt[:, hs, :])
                            else:
                                nc.vector.tensor_tensor(
                                    out=den[hp][:, :, vis0:S],
                                    in0=den[hp][:, :, vis0:S],
                                    in1=pt[:, hs, vis0:S],
                                    op=mybir.AluOpType.add)
                            # columns [128(kt-NKC) : 128(kt-NKC+1)] final now
                            if NKC <= kt < KT - 1:
                                f0 = 128 * (kt - NKC)
                                nc.vector.tensor_copy(
                                    out=denb[hp][:, :, f0:f0 + 128],
                                    in_=den[hp][:, :, f0:f0 + 128])

                    def pv_step(kt):
                        vis0 = 128 * (kt - NKC) if kt >= NKC else 0
                        pt = pt_tiles.pop(kt)
                        for h in range(NH):
                            o = pv[h][:, :] if kt == 0 else pv[h][:, vis0:S]
                            nc.tensor.matmul(o, v_b[:, kt, :], pt[:, h, vis0:S],
                                             start=(kt == 0), stop=(kt == KT - 1))

                    SPL = 128 * (KT - 1 - NKC)  # cols final before last kt
                    psdb_t = {}

                    rec_t = {}

                    def recip_head(h):
                        """recb[h] = 1/sum_keys(exp) via broadcast-matmul +
                        exp(-ln(x)) on the Scalar engine — ln/exp share one
                        activation table set with the softmax exps, keeping
                        the slow DVE reciprocal off the critical path. Head
                        pairs share one 2-bank PSUM tile (ln in place)."""
                        hp = h // 2
                        if hp not in psdb_t:
                            psdb_t[hp] = ps.tile([128, 2, S], F32, tag="ps",
                                                 name=f"psdb{b}_{hp}")
                        ps_db = psdb_t[hp]
                        bank = h % 2
                        nc.tensor.matmul(ps_db[:, bank, :], ones_m[:, :],
                                         denb[hp][:, bank, :],
                                         start=True, stop=True)
                        nc.scalar.activation(
                            ps_db[:, bank, :], ps_db[:, bank, :],
                            mybir.ActivationFunctionType.Ln)
                        recb = atp.tile([128, S], F32, tag="recb",
                                         name=f"recb{b}_{h}")
                        nc.scalar.activation(
                            recb[:, :], ps_db[:, bank, :],
                            mybir.ActivationFunctionType.Exp, scale=-1.0)
                        rec_t[h] = recb

                    def finish_g(g):
                        for h in range(hpg * g, hpg * g + hpg):
                            at = atp.tile([128, S], BF16, tag="at")
                            nc.vector.tensor_tensor(
                                out=at[:, :], in0=pv[h][:, :],
                                in1=rec_t.pop(h)[:, :],
                                op=mybir.AluOpType.mult)
                            hh = h - hpg * g
                            nc.sync.dma_start(
                                out=ag_in[b][g][128 * hh:128 * hh + 128, :],
                                in_=at[:, :])
                        for _d in range(2 if DUP_COLL else 1):
                            nc.gpsimd.collective_compute(
                                "AllGather", mybir.AluOpType.bypass,
                                replica_groups=rg,
                                ins=[ag_in[b][g][:, :]], outs=[ag_out[b][g][:, :]])

                    for kt in range(KT):
                        scores(kt)
                        if kt >= 2:
                            pv_step(kt - 2)
                    pv_step(KT - 2)
                    pv_step(KT - 1)
                    # final bf16 den slices for both head pairs, ahead of the
                    # per-head normalize chains so the den matmuls can stream
                    for hp in range(2):
                        nc.vector.tensor_copy(out=denb[hp][:, :, SPL:S],
                                              in_=den[hp][:, :, SPL:S])
                    # interleave head pairs so consecutive den matmuls hit
                    # different PSUM tiles (no write-after-read stall)
                    for h in ((0, 2, 1, 3) if NH == 4 else range(NH)):
                        recip_head(h)
                    for g in range(ngr):
                        finish_g(g)

                    # ---- output projection, PIPE_DEPTH blocks behind ----
                    pending.append(b)
                    if len(pending) > PIPE_DEPTH:
                        emit_wo(pending.pop(0))
            for pb in pending:
                emit_wo(pb)

    _dedup_ldweights(nc)
    _split_excess_waits(nc)
    return nc


_nc_cache = {}


def _get_nc(iters: int):
    if iters not in _nc_cache:
        _nc_cache[iters] = build_nc(iters)
    return _nc_cache[iters]


def make_in_maps(x, wq, wk, wv, wo, freqs_cos, freqs_sin, cache_k, cache_v):
    bf = lambda a: np.ascontiguousarray(a).astype(NPBF16)
    xT = bf(x.reshape(NT, D).T)
    cosT = bf(freqs_cos.T)
    sinT = bf(freqs_sin.T)
    # permute rope pair dims to [evens, odds] within each head
    wq_p = wq.reshape(H, HD, D)[:, PERM, :].reshape(H * HD, D)
    wk_p = wk.reshape(HKV, HD, D)[:, PERM, :].reshape(HKV * HD, D)
    in_maps = []
    for c in range(NCORES):
        in_maps.append({
            "xT": xT,
            "wqT": bf(wq_p[EL * c:EL * (c + 1), :].T),
            "wkT": bf(wk_p[HD * c:HD * (c + 1), :].T),
            "wvT": bf(wv[HD * c:HD * (c + 1), :].T),
            "woT": bf(wo[EL * c:EL * (c + 1), :].T),
            "ckT": bf(cache_k[:, :, c, :].transpose(0, 2, 1)[:, PERM, :]),
            "cv": bf(cache_v[:, :, c, :]),
            "cosT": cosT, "sinT": sinT,
        })
    return in_maps


def assemble_out(results):
    return np.concatenate(
        [results[c]["out"].T for c in range(NCORES)], axis=1
    ).reshape(B, S, D)


def kernel(x, wq, wk, wv, wo, freqs_cos, freqs_sin, cache_k, cache_v,
           start_pos=START, **_ignored):
    assert x.shape == (B, S, D) and int(start_pos) == START
    nc = _get_nc(1)
    in_maps = make_in_maps(np.asarray(x, np.float32), np.asarray(wq, np.float32),
                           np.asarray(wk, np.float32), np.asarray(wv, np.float32),
                           np.asarray(wo, np.float32),
                           np.asarray(freqs_cos, np.float32),
                           np.asarray(freqs_sin, np.float32),
                           np.asarray(cache_k, np.float32),
                           np.asarray(cache_v, np.float32))
    res = run_bass_kernel_spmd(nc, in_maps, core_ids=list(range(NCORES)),
                               trace=False)
    return assemble_out(res.results)

